# revision 15
# baseline (speedup 1.0000x reference)
"""Trainium2 Bass kernel for DLLinearZeroDiagonal:
    y = x @ W.T + bias,  W = zero-diagonal 4096x4096 with strict triangles
    packed row-major in upper_w / lower_w.

Strategy (8 NeuronCores):
  - 2-way shard over output dim (o) x 4-way shard over batch (b).
  - Host reconstructs the dense weight (sanctioned by the sharding hint:
    "replicate the reconstructed weight") and lays out W^T / x^T shards in
    the tile order the device DMAs want.  All FLOPs + bias happen on device.
  - Per core: resident x^T shard in SBUF, stream W^T slabs once,
    1024 accumulating matmuls (128x128 @ 128x512), bias add on DVE,
    outputs written as y^T shard and untransposed on host.

Variants (KVARIANT env var, default the current best):
  resident2     fp32r everywhere (baseline, 169.4 us)
  bf16          bf16 x/W/y (284.8 us -- PE-slow, kept for reference)
  probe_pe_*    one W slab reused -> measures PE rate with tiny DMA
  probe_dma_*   no matmuls -> measures pure DMA time for the byte set
"""

import os
import numpy as np

N = 4096            # in/out feature dim and batch
RO, RB = 2, 4       # shard ways over output-dim / batch
OC = N // RO        # 2048 output cols per core
BC = N // RB        # 1024 batch rows per core
NW = OC // 128      # 16 stationary o-blocks per core
NT = N // 128       # 32 contraction tiles
NN = BC // 512      # 2 moving b-tiles per core

VARIANT = os.environ.get("KVARIANT", "resident2")

_PROGRAM = None


def _variant_opts(variant):
    dtype = "bf16" if "bf16" in variant else "f32r"
    pe_probe = variant.startswith("probe_pe")
    dma_probe = variant.startswith("probe_dma")
    return dtype, pe_probe, dma_probe


def _build_program_bf16v2(reps=None):
    """bf16 + LDW dedupe + double-buffered resident x + y on sync ring."""
    import concourse.bacc as bacc
    import concourse.bass as bass
    import concourse.tile as tile
    from concourse import mybir
    from contextlib import ExitStack, nullcontext

    F32 = mybir.dt.float32
    BF16 = mybir.dt.bfloat16

    nc = bacc.Bacc("TRN2", target_bir_lowering=False, debug=False)
    xt = nc.dram_tensor("xt", [NT, 128, BC], BF16, kind="ExternalInput")
    wt = nc.dram_tensor("wt", [NW, 128, NT, 128], BF16, kind="ExternalInput")
    bias = nc.dram_tensor("bias", [128, NW], F32, kind="ExternalInput")
    yt = nc.dram_tensor("yt", [OC, BC], BF16, kind="ExternalOutput")

    with tile.TileContext(nc) as tc, ExitStack() as ctx:
        xtp = ctx.enter_context(tc.tile_pool(name="xtp", bufs=2))
        wtp = ctx.enter_context(tc.tile_pool(name="wtp", bufs=2))
        bp = ctx.enter_context(tc.tile_pool(name="bp", bufs=1))
        op = ctx.enter_context(tc.tile_pool(name="op", bufs=4))
        pp = ctx.enter_context(tc.tile_pool(name="pp", bufs=8, space="PSUM"))

        loop = tc.For_i(0, reps, 1) if reps is not None else nullcontext()
        with loop:
            # resident x^T shard, one DMA: [128, NT*BC], col block t = j 128t+p
            xt_res = xtp.tile([128, NT * BC], BF16)
            nc.scalar.dma_start(
                xt_res[:],
                bass.AP(xt, 0, [[BC, 128], [128 * BC, NT], [1, BC]]),
            )
            bias_sb = bp.tile([128, NW], F32)
            nc.sync.dma_start(bias_sb[:], bass.AP(bias, 0, [[NW, 128], [1, NW]]))

            for w in range(NW):
                slab = wtp.tile([128, NT * 128], BF16)
                nc.sync.dma_start(
                    slab[:],
                    bass.AP(wt, w * 128 * NT * 128,
                            [[NT * 128, 128], [1, NT * 128]]),
                )
                psums = [pp.tile([128, 512], F32, name=f"ps{n}", tag="ps")
                         for n in range(NN)]
                for t in range(NT):
                    lhsT = slab[:, t * 128:(t + 1) * 128]
                    for n in range(NN):
                        nc.tensor.matmul(
                            psums[n][:],
                            lhsT,
                            xt_res[:, t * BC + n * 512: t * BC + n * 512 + 512],
                            start=(t == 0),
                            stop=(t == NT - 1),
                        )
                for n in range(NN):
                    ot = op.tile([128, 512], BF16)
                    nc.vector.tensor_scalar_add(ot[:], psums[n][:],
                                                bias_sb[:, w:w + 1])
                    nc.sync.dma_start(
                        bass.AP(yt, w * 128 * BC + n * 512, [[BC, 128], [1, 512]]),
                        ot[:],
                    )
    n_removed = _dedupe_ldweights(nc)
    assert n_removed >= 256, f"LDW dedupe removed only {n_removed}"
    nc.compile()
    return nc


def _build_program_bf16v4(reps=None):
    """bf16 v1 + double-buffered resident x (idle-hides the x reload)."""
    import concourse.bacc as bacc
    import concourse.bass as bass
    import concourse.tile as tile
    from concourse import mybir
    from contextlib import ExitStack, nullcontext

    F32 = mybir.dt.float32
    BF16 = mybir.dt.bfloat16

    nc = bacc.Bacc("TRN2", target_bir_lowering=False, debug=False)
    xt = nc.dram_tensor("xt", [NT, 128, BC], BF16, kind="ExternalInput")
    wt = nc.dram_tensor("wt", [NW, 128, NT, 128], BF16, kind="ExternalInput")
    bias = nc.dram_tensor("bias", [128, NW], F32, kind="ExternalInput")
    yt = nc.dram_tensor("yt", [OC, BC], BF16, kind="ExternalOutput")

    with tile.TileContext(nc) as tc, ExitStack() as ctx:
        xtp = ctx.enter_context(tc.tile_pool(name="xtp", bufs=2))
        wtp = ctx.enter_context(tc.tile_pool(name="wtp", bufs=2))
        bp = ctx.enter_context(tc.tile_pool(name="bp", bufs=1))
        op = ctx.enter_context(tc.tile_pool(name="op", bufs=4))
        pp = ctx.enter_context(tc.tile_pool(name="pp", bufs=8, space="PSUM"))

        loop = tc.For_i(0, reps, 1) if reps is not None else nullcontext()
        with loop:
            xt_res = xtp.tile([128, NT * BC], BF16)
            for t in range(NT):
                nc.scalar.dma_start(
                    xt_res[:, t * BC:(t + 1) * BC],
                    bass.AP(xt, t * 128 * BC, [[BC, 128], [1, BC]]),
                )
            bias_sb = bp.tile([128, NW], F32)
            nc.sync.dma_start(bias_sb[:], bass.AP(bias, 0, [[NW, 128], [1, NW]]))

            for w in range(NW):
                slab = wtp.tile([128, NT * 128], BF16)
                nc.sync.dma_start(
                    slab[:],
                    bass.AP(wt, w * 128 * NT * 128,
                            [[NT * 128, 128], [1, NT * 128]]),
                )
                psums = [pp.tile([128, 512], F32, name=f"ps{n}", tag="ps")
                         for n in range(NN)]
                for t in range(NT):
                    lhsT = slab[:, t * 128:(t + 1) * 128]
                    for n in range(NN):
                        nc.tensor.matmul(
                            psums[n][:],
                            lhsT,
                            xt_res[:, t * BC + n * 512: t * BC + n * 512 + 512],
                            start=(t == 0),
                            stop=(t == NT - 1),
                        )
                for n in range(NN):
                    ot = op.tile([128, 512], BF16)
                    nc.vector.tensor_scalar_add(ot[:], psums[n][:],
                                                bias_sb[:, w:w + 1])
                    nc.scalar.dma_start(
                        bass.AP(yt, w * 128 * BC + n * 512, [[BC, 128], [1, 512]]),
                        ot[:],
                    )
    nc.compile()
    return nc


def _build_program_bf16v3(reps=None):
    """Exactly the bf16 v1 structure + LDW dedupe (isolates dedupe effect)."""
    import concourse.bacc as bacc
    import concourse.bass as bass
    import concourse.tile as tile
    from concourse import mybir
    from contextlib import ExitStack, nullcontext

    F32 = mybir.dt.float32
    BF16 = mybir.dt.bfloat16

    nc = bacc.Bacc("TRN2", target_bir_lowering=False, debug=False)
    xt = nc.dram_tensor("xt", [NT, 128, BC], BF16, kind="ExternalInput")
    wt = nc.dram_tensor("wt", [NW, 128, NT, 128], BF16, kind="ExternalInput")
    bias = nc.dram_tensor("bias", [128, NW], F32, kind="ExternalInput")
    yt = nc.dram_tensor("yt", [OC, BC], BF16, kind="ExternalOutput")

    with tile.TileContext(nc) as tc, ExitStack() as ctx:
        xtp = ctx.enter_context(tc.tile_pool(name="xtp", bufs=1))
        wtp = ctx.enter_context(tc.tile_pool(name="wtp", bufs=2))
        bp = ctx.enter_context(tc.tile_pool(name="bp", bufs=1))
        op = ctx.enter_context(tc.tile_pool(name="op", bufs=4))
        pp = ctx.enter_context(tc.tile_pool(name="pp", bufs=8, space="PSUM"))

        loop = tc.For_i(0, reps, 1) if reps is not None else nullcontext()
        with loop:
            xt_res = xtp.tile([128, NT * BC], BF16)
            for t in range(NT):
                nc.scalar.dma_start(
                    xt_res[:, t * BC:(t + 1) * BC],
                    bass.AP(xt, t * 128 * BC, [[BC, 128], [1, BC]]),
                )
            bias_sb = bp.tile([128, NW], F32)
            nc.sync.dma_start(bias_sb[:], bass.AP(bias, 0, [[NW, 128], [1, NW]]))

            for w in range(NW):
                slab = wtp.tile([128, NT * 128], BF16)
                nc.sync.dma_start(
                    slab[:],
                    bass.AP(wt, w * 128 * NT * 128,
                            [[NT * 128, 128], [1, NT * 128]]),
                )
                psums = [pp.tile([128, 512], F32, name=f"ps{n}", tag="ps")
                         for n in range(NN)]
                for t in range(NT):
                    lhsT = slab[:, t * 128:(t + 1) * 128]
                    for n in range(NN):
                        nc.tensor.matmul(
                            psums[n][:],
                            lhsT,
                            xt_res[:, t * BC + n * 512: t * BC + n * 512 + 512],
                            start=(t == 0),
                            stop=(t == NT - 1),
                        )
                for n in range(NN):
                    ot = op.tile([128, 512], BF16)
                    nc.vector.tensor_scalar_add(ot[:], psums[n][:],
                                                bias_sb[:, w:w + 1])
                    nc.scalar.dma_start(
                        bass.AP(yt, w * 128 * BC + n * 512, [[BC, 128], [1, 512]]),
                        ot[:],
                    )
    _dedupe_ldweights(nc)
    nc.compile()
    return nc


def _build_probe_mm(reps=None, dtype="bf16", quad=False):
    """Pure-MM probe: x resident + ONE W slab, full 1024-MM schedule, no
    bias/output work (one token y write).  quad=True halves the LDW count
    by issuing 4 MMs per stationary (t range halved, rhs slices reused)."""
    import concourse.bacc as bacc
    import concourse.bass as bass
    import concourse.tile as tile
    from concourse import mybir
    from contextlib import ExitStack, nullcontext

    F32 = mybir.dt.float32
    MMD = mybir.dt.bfloat16 if dtype == "bf16" else mybir.dt.float32r
    OUTD = mybir.dt.bfloat16 if dtype == "bf16" else mybir.dt.float32

    nc = bacc.Bacc("TRN2", target_bir_lowering=False, debug=False)
    xt = nc.dram_tensor("xt", [NT, 128, BC], MMD, kind="ExternalInput")
    wt = nc.dram_tensor("wt", [NW, 128, NT, 128], MMD, kind="ExternalInput")
    bias = nc.dram_tensor("bias", [128, NW], F32, kind="ExternalInput")
    yt = nc.dram_tensor("yt", [OC, BC], OUTD, kind="ExternalOutput")

    if quad == "n256":
        NTP, NNP, NMOV = NT, 4, 256        # 2048 MMs of 256 cols
    elif quad:
        NTP, NNP, NMOV = NT // 2, 4, 512   # quad: half the t range
    else:
        NTP, NNP, NMOV = NT, NN, 512

    with tile.TileContext(nc) as tc, ExitStack() as ctx:
        xtp = ctx.enter_context(tc.tile_pool(name="xtp", bufs=1))
        wtp = ctx.enter_context(tc.tile_pool(name="wtp", bufs=1))
        bp = ctx.enter_context(tc.tile_pool(name="bp", bufs=1))
        op = ctx.enter_context(tc.tile_pool(name="op", bufs=1))
        pp = ctx.enter_context(tc.tile_pool(name="pp", bufs=8, space="PSUM"))

        loop = tc.For_i(0, reps, 1) if reps is not None else nullcontext()
        with loop:
            xt_res = xtp.tile([128, NT * BC], MMD)
            for t in range(NT):
                nc.scalar.dma_start(
                    xt_res[:, t * BC:(t + 1) * BC],
                    bass.AP(xt, t * 128 * BC, [[BC, 128], [1, BC]]),
                )
            bias_sb = bp.tile([128, NW], F32)
            nc.sync.dma_start(bias_sb[:], bass.AP(bias, 0, [[NW, 128], [1, NW]]))
            slab = wtp.tile([128, NT * 128], MMD)
            nc.sync.dma_start(
                slab[:], bass.AP(wt, 0, [[NT * 128, 128], [1, NT * 128]]))

            last_ps = None
            for w in range(NW):
                psums = [pp.tile([128, NMOV], F32, name=f"ps{n}", tag="ps")
                         for n in range(NNP)]
                for t in range(NTP):
                    lhsT = slab[:, t * 128:(t + 1) * 128]
                    for n in range(NNP):
                        col = (t * BC + (n * NMOV) % BC)
                        nc.tensor.matmul(
                            psums[n][:], lhsT, xt_res[:, col:col + NMOV],
                            start=(t == 0), stop=(t == NTP - 1),
                        )
                last_ps = psums[0]
            ot = op.tile([128, NMOV], OUTD)
            nc.vector.tensor_scalar_add(ot[:], last_ps[:], bias_sb[:, 0:1])
            nc.scalar.dma_start(
                bass.AP(yt, 0, [[BC, 128], [1, NMOV]]), ot[:])
    nc.compile()
    return nc


def _dedupe_ldweights(nc):
    """Remove redundant InstLdweights: tile_legalize emits one per matmul,
    but consecutive matmuls sharing the same stationary weights (our n=0/1
    pairs) only need the first.  The PE array retains the stationary operand,
    so an Ldweights identical (same physical AP) to the previous one on the
    PE queue -- with only Matmults in between and no semaphore waits of its
    own -- is a no-op that still costs ~128 serial cycles on HW."""
    from concourse import mybir

    removed = [0]

    def walk(block):
        insts = block.instructions
        last_sig = [None]

        keep = []
        for inst in insts:
            nm = type(inst).__name__
            if nm == "InstLdweights":
                si = inst.sync_info
                has_wait = si is not None and len(si.on_wait) > 0
                has_update = si is not None and len(si.on_update) > 0
                sig = (repr(inst.ins[0]), str(inst.perf_mode),
                       str(inst.is_transpose))
                if (not has_wait and not has_update
                        and sig == last_sig[0]):
                    removed[0] += 1
                    continue          # drop redundant reload
                last_sig[0] = sig
            elif nm == "InstMatmult":
                pass                  # keeps the loaded weights
            elif nm in ("InstEventSemaphore",):
                pass                  # sem-only, does not touch PE array
            else:
                last_sig[0] = None    # anything else: be conservative
            keep.append(inst)
            for sub in getattr(inst, "blocks", []) or []:
                walk(sub)
        if len(keep) != len(insts):
            while len(insts):
                insts.pop()
            for i in keep:
                insts.append(i)

    for b in nc.m.functions[0].blocks:
        walk(b)
    return removed[0]


def _build_program(reps=None, variant=None):
    if variant is None:
        variant = VARIANT
    if variant.startswith("probe_mm"):
        q = "n256" if "n256" in variant else ("quad" in variant)
        return _build_probe_mm(reps, dtype="bf16" if variant.endswith("bf16")
                               else "f32r", quad=q)
    if variant.startswith("bf16v2"):
        return _build_program_bf16v2(reps)
    if variant.startswith("bf16v3"):
        return _build_program_bf16v3(reps)
    if variant.startswith("bf16v4"):
        return _build_program_bf16v4(reps)
    dtype, pe_probe, dma_probe = _variant_opts(variant)

    import concourse.bacc as bacc
    import concourse.bass as bass
    import concourse.tile as tile
    from concourse import mybir
    from contextlib import ExitStack, nullcontext

    F32 = mybir.dt.float32
    MMD = mybir.dt.bfloat16 if dtype == "bf16" else mybir.dt.float32r
    OUTD = mybir.dt.bfloat16 if dtype == "bf16" else mybir.dt.float32

    nc = bacc.Bacc("TRN2", target_bir_lowering=False, debug=False)
    # host-tiled layouts (see _shard_inputs):
    #   xt[t, p, b]     = x[b0+b, 128t+p]
    #   wt[w, p, t, o'] = W[o0+128w+o', 128t+p]
    #   bias2[p, w]     = bias[o0+128w+p]
    xt = nc.dram_tensor("xt", [NT, 128, BC], MMD, kind="ExternalInput")
    wt = nc.dram_tensor("wt", [NW, 128, NT, 128], MMD, kind="ExternalInput")
    bias = nc.dram_tensor("bias", [128, NW], F32, kind="ExternalInput")
    yt = nc.dram_tensor("yt", [OC, BC], OUTD, kind="ExternalOutput")

    with tile.TileContext(nc) as tc, ExitStack() as ctx:
        xtp = ctx.enter_context(tc.tile_pool(name="xtp", bufs=1))
        wtp = ctx.enter_context(tc.tile_pool(name="wtp", bufs=2))
        bp = ctx.enter_context(tc.tile_pool(name="bp", bufs=1))
        op = ctx.enter_context(tc.tile_pool(name="op", bufs=4))
        pp = ctx.enter_context(tc.tile_pool(name="pp", bufs=8, space="PSUM"))

        loop = tc.For_i(0, reps, 1) if reps is not None else nullcontext()
        with loop:
            # resident x^T shard: [128, NT*BC] ; column block t holds j=128t+p
            xt_res = xtp.tile([128, NT * BC], MMD)
            for t in range(NT):
                nc.scalar.dma_start(
                    xt_res[:, t * BC:(t + 1) * BC],
                    bass.AP(xt, t * 128 * BC, [[BC, 128], [1, BC]]),
                )
            bias_sb = bp.tile([128, NW], F32)
            nc.sync.dma_start(bias_sb[:], bass.AP(bias, 0, [[NW, 128], [1, NW]]))

            shared_slab = None
            if pe_probe:
                shared_slab = wtp.tile([128, NT * 128], MMD, name="slab0")
                nc.sync.dma_start(
                    shared_slab[:],
                    bass.AP(wt, 0, [[NT * 128, 128], [1, NT * 128]]),
                )

            for w in range(NW):
                if pe_probe:
                    slab = shared_slab
                else:
                    slab = wtp.tile([128, NT * 128], MMD)
                    nc.sync.dma_start(
                        slab[:],
                        bass.AP(wt, w * 128 * NT * 128,
                                [[NT * 128, 128], [1, NT * 128]]),
                    )
                if dma_probe:
                    continue
                psums = [pp.tile([128, 512], F32, name=f"ps{n}", tag="ps")
                         for n in range(NN)]
                for t in range(NT):
                    lhsT = slab[:, t * 128:(t + 1) * 128]
                    for n in range(NN):
                        nc.tensor.matmul(
                            psums[n][:],
                            lhsT,
                            xt_res[:, t * BC + n * 512: t * BC + n * 512 + 512],
                            start=(t == 0),
                            stop=(t == NT - 1),
                        )
                for n in range(NN):
                    ot = op.tile([128, 512], OUTD)
                    nc.vector.tensor_scalar_add(ot[:], psums[n][:],
                                                bias_sb[:, w:w + 1])
                    nc.scalar.dma_start(
                        bass.AP(yt, w * 128 * BC + n * 512, [[BC, 128], [1, 512]]),
                        ot[:],
                    )
            if dma_probe:
                # keep the output-write traffic: DMA zeros-ish tiles out
                for w in range(NW):
                    for n in range(NN):
                        ot = op.tile([128, 512], OUTD)
                        nc.vector.memset(ot[:], 0.0)
                        nc.scalar.dma_start(
                            bass.AP(yt, w * 128 * BC + n * 512,
                                    [[BC, 128], [1, 512]]),
                            ot[:],
                        )
    nc.compile()
    return nc


def _get_program():
    global _PROGRAM
    if _PROGRAM is None:
        _PROGRAM = _build_program()
    return _PROGRAM


def _reconstruct_wt(upper_w: np.ndarray, lower_w: np.ndarray) -> np.ndarray:
    """Dense W [o, j] from the packed strict triangles (row-major fill)."""
    W = np.zeros((N, N), dtype=np.float32)
    iu = np.triu_indices(N, k=1)
    il = np.tril_indices(N, k=-1)
    W[iu] = upper_w
    W[il] = lower_w
    return W


def _shard_inputs(x, upper_w, lower_w, bias, variant=None):
    if variant is None:
        variant = VARIANT
    dtype, _, _ = _variant_opts(variant)
    x = np.asarray(x, dtype=np.float32)
    upper_w = np.asarray(upper_w, dtype=np.float32)
    lower_w = np.asarray(lower_w, dtype=np.float32)
    bias = np.asarray(bias, dtype=np.float32)

    W = _reconstruct_wt(upper_w, lower_w)

    if dtype == "bf16":
        import ml_dtypes
        mmdt = ml_dtypes.bfloat16
    else:
        mmdt = np.float32

    wt_shards = []
    bias_shards = []
    for ob in range(RO):
        Ws = W[ob * OC:(ob + 1) * OC, :]                       # [OC o, N j]
        # wt[w, p, t, o'] = Ws[128w+o', 128t+p]
        wts = np.ascontiguousarray(
            Ws.T.reshape(NT, 128, NW, 128).transpose(2, 1, 0, 3).astype(mmdt)
        )
        wt_shards.append(wts)
        bias_shards.append(
            np.ascontiguousarray(bias[ob * OC:(ob + 1) * OC].reshape(NW, 128).T)
        )

    xt_shards = []
    for bb in range(RB):
        xs = x[bb * BC:(bb + 1) * BC, :]                       # [BC b, N j]
        xt_shards.append(
            np.ascontiguousarray(xs.T.reshape(NT, 128, BC).astype(mmdt)))

    in_maps = []
    for c in range(8):
        ob, bb = c // RB, c % RB
        in_maps.append({
            "xt": xt_shards[bb],
            "wt": wt_shards[ob],
            "bias": bias_shards[ob],
        })
    return in_maps


def _assemble(results) -> np.ndarray:
    y = np.empty((N, N), dtype=np.float32)
    for c in range(8):
        ob, bb = c // RB, c % RB
        y[bb * BC:(bb + 1) * BC, ob * OC:(ob + 1) * OC] = \
            results[c]["yt"].T.astype(np.float32)
    return y


def kernel(x, upper_w, lower_w, bias):
    from concourse import bass_utils

    nc = _get_program()
    in_maps = _shard_inputs(x, upper_w, lower_w, bias)
    res = bass_utils.run_bass_kernel_spmd(nc, in_maps, core_ids=list(range(8)))
    return _assemble(res.results)


# revision 17
# speedup vs baseline: 1.0953x; 1.0953x over previous
"""Trainium2 Bass kernel for DLLinearZeroDiagonal:
    y = x @ W.T + bias,  W = zero-diagonal 4096x4096 with strict triangles
    packed row-major in upper_w / lower_w.

Strategy (8 NeuronCores):
  - 2-way shard over output dim (o) x 4-way shard over batch (b).
  - Host reconstructs the dense weight (sanctioned by the sharding hint:
    "replicate the reconstructed weight") and lays out W^T / x^T shards in
    the tile order the device DMAs want.  All FLOPs + bias happen on device.
  - Per core: resident x^T shard in SBUF, stream W^T slabs once,
    1024 accumulating matmuls (128x128 @ 128x512), bias add on DVE,
    outputs written as y^T shard and untransposed on host.

Measured (257-rep on-device loop, median-differenced, this machine):
  resident2 fp32r: 311.7 us/core   bf16 (default): 284.8 us/core
  pure-MM probe floor: 279.5 us/core -- the PE streams 1 column/cycle at
  an effective ~1.87 GHz regardless of dtype (fp32r==bf16==N256/N512),
  so 524288 moving columns/core is compute-bound at ~280 us.  bf16 halves
  HBM traffic (56.6 -> 29.4 MB/core) and beats fp32r's 4-byte LDW path.
  fp8 DoubleRow (2 cols/cycle) fails the 2e-2 gate (sigma ~5%); LDW
  dedupe, x double-buffering, ring re-balancing all measured neutral-to-
  worse than this v1 structure.

Variants (KVARIANT env var, default the current best):
  bf16          bf16 x/W/y, fp32 psum+bias (DEFAULT, 284.8 us)
  resident2     fp32r everywhere (original baseline)
  bf16v2..v4    perturbation experiments (all slower; kept for reference)
  probe_*       PE/DMA isolation microbenchmarks
"""

import os
import numpy as np

N = 4096            # in/out feature dim and batch
RO, RB = 2, 4       # shard ways over output-dim / batch
OC = N // RO        # 2048 output cols per core
BC = N // RB        # 1024 batch rows per core
NW = OC // 128      # 16 stationary o-blocks per core
NT = N // 128       # 32 contraction tiles
NN = BC // 512      # 2 moving b-tiles per core

VARIANT = os.environ.get("KVARIANT", "bf16")

_PROGRAM = None


def _variant_opts(variant):
    dtype = "bf16" if "bf16" in variant else "f32r"
    pe_probe = variant.startswith("probe_pe")
    dma_probe = variant.startswith("probe_dma")
    return dtype, pe_probe, dma_probe


def _build_program_bf16v2(reps=None):
    """bf16 + LDW dedupe + double-buffered resident x + y on sync ring."""
    import concourse.bacc as bacc
    import concourse.bass as bass
    import concourse.tile as tile
    from concourse import mybir
    from contextlib import ExitStack, nullcontext

    F32 = mybir.dt.float32
    BF16 = mybir.dt.bfloat16

    nc = bacc.Bacc("TRN2", target_bir_lowering=False, debug=False)
    xt = nc.dram_tensor("xt", [NT, 128, BC], BF16, kind="ExternalInput")
    wt = nc.dram_tensor("wt", [NW, 128, NT, 128], BF16, kind="ExternalInput")
    bias = nc.dram_tensor("bias", [128, NW], F32, kind="ExternalInput")
    yt = nc.dram_tensor("yt", [OC, BC], BF16, kind="ExternalOutput")

    with tile.TileContext(nc) as tc, ExitStack() as ctx:
        xtp = ctx.enter_context(tc.tile_pool(name="xtp", bufs=2))
        wtp = ctx.enter_context(tc.tile_pool(name="wtp", bufs=2))
        bp = ctx.enter_context(tc.tile_pool(name="bp", bufs=1))
        op = ctx.enter_context(tc.tile_pool(name="op", bufs=4))
        pp = ctx.enter_context(tc.tile_pool(name="pp", bufs=8, space="PSUM"))

        loop = tc.For_i(0, reps, 1) if reps is not None else nullcontext()
        with loop:
            # resident x^T shard, one DMA: [128, NT*BC], col block t = j 128t+p
            xt_res = xtp.tile([128, NT * BC], BF16)
            nc.scalar.dma_start(
                xt_res[:],
                bass.AP(xt, 0, [[BC, 128], [128 * BC, NT], [1, BC]]),
            )
            bias_sb = bp.tile([128, NW], F32)
            nc.sync.dma_start(bias_sb[:], bass.AP(bias, 0, [[NW, 128], [1, NW]]))

            for w in range(NW):
                slab = wtp.tile([128, NT * 128], BF16)
                nc.sync.dma_start(
                    slab[:],
                    bass.AP(wt, w * 128 * NT * 128,
                            [[NT * 128, 128], [1, NT * 128]]),
                )
                psums = [pp.tile([128, 512], F32, name=f"ps{n}", tag="ps")
                         for n in range(NN)]
                for t in range(NT):
                    lhsT = slab[:, t * 128:(t + 1) * 128]
                    for n in range(NN):
                        nc.tensor.matmul(
                            psums[n][:],
                            lhsT,
                            xt_res[:, t * BC + n * 512: t * BC + n * 512 + 512],
                            start=(t == 0),
                            stop=(t == NT - 1),
                        )
                for n in range(NN):
                    ot = op.tile([128, 512], BF16)
                    nc.vector.tensor_scalar_add(ot[:], psums[n][:],
                                                bias_sb[:, w:w + 1])
                    nc.sync.dma_start(
                        bass.AP(yt, w * 128 * BC + n * 512, [[BC, 128], [1, 512]]),
                        ot[:],
                    )
    n_removed = _dedupe_ldweights(nc)
    assert n_removed >= 256, f"LDW dedupe removed only {n_removed}"
    nc.compile()
    return nc


def _build_program_bf16v4(reps=None):
    """bf16 v1 + double-buffered resident x (idle-hides the x reload)."""
    import concourse.bacc as bacc
    import concourse.bass as bass
    import concourse.tile as tile
    from concourse import mybir
    from contextlib import ExitStack, nullcontext

    F32 = mybir.dt.float32
    BF16 = mybir.dt.bfloat16

    nc = bacc.Bacc("TRN2", target_bir_lowering=False, debug=False)
    xt = nc.dram_tensor("xt", [NT, 128, BC], BF16, kind="ExternalInput")
    wt = nc.dram_tensor("wt", [NW, 128, NT, 128], BF16, kind="ExternalInput")
    bias = nc.dram_tensor("bias", [128, NW], F32, kind="ExternalInput")
    yt = nc.dram_tensor("yt", [OC, BC], BF16, kind="ExternalOutput")

    with tile.TileContext(nc) as tc, ExitStack() as ctx:
        xtp = ctx.enter_context(tc.tile_pool(name="xtp", bufs=2))
        wtp = ctx.enter_context(tc.tile_pool(name="wtp", bufs=2))
        bp = ctx.enter_context(tc.tile_pool(name="bp", bufs=1))
        op = ctx.enter_context(tc.tile_pool(name="op", bufs=4))
        pp = ctx.enter_context(tc.tile_pool(name="pp", bufs=8, space="PSUM"))

        loop = tc.For_i(0, reps, 1) if reps is not None else nullcontext()
        with loop:
            xt_res = xtp.tile([128, NT * BC], BF16)
            for t in range(NT):
                nc.scalar.dma_start(
                    xt_res[:, t * BC:(t + 1) * BC],
                    bass.AP(xt, t * 128 * BC, [[BC, 128], [1, BC]]),
                )
            bias_sb = bp.tile([128, NW], F32)
            nc.sync.dma_start(bias_sb[:], bass.AP(bias, 0, [[NW, 128], [1, NW]]))

            for w in range(NW):
                slab = wtp.tile([128, NT * 128], BF16)
                nc.sync.dma_start(
                    slab[:],
                    bass.AP(wt, w * 128 * NT * 128,
                            [[NT * 128, 128], [1, NT * 128]]),
                )
                psums = [pp.tile([128, 512], F32, name=f"ps{n}", tag="ps")
                         for n in range(NN)]
                for t in range(NT):
                    lhsT = slab[:, t * 128:(t + 1) * 128]
                    for n in range(NN):
                        nc.tensor.matmul(
                            psums[n][:],
                            lhsT,
                            xt_res[:, t * BC + n * 512: t * BC + n * 512 + 512],
                            start=(t == 0),
                            stop=(t == NT - 1),
                        )
                for n in range(NN):
                    ot = op.tile([128, 512], BF16)
                    nc.vector.tensor_scalar_add(ot[:], psums[n][:],
                                                bias_sb[:, w:w + 1])
                    nc.scalar.dma_start(
                        bass.AP(yt, w * 128 * BC + n * 512, [[BC, 128], [1, 512]]),
                        ot[:],
                    )
    nc.compile()
    return nc


def _build_program_bf16v3(reps=None):
    """Exactly the bf16 v1 structure + LDW dedupe (isolates dedupe effect)."""
    import concourse.bacc as bacc
    import concourse.bass as bass
    import concourse.tile as tile
    from concourse import mybir
    from contextlib import ExitStack, nullcontext

    F32 = mybir.dt.float32
    BF16 = mybir.dt.bfloat16

    nc = bacc.Bacc("TRN2", target_bir_lowering=False, debug=False)
    xt = nc.dram_tensor("xt", [NT, 128, BC], BF16, kind="ExternalInput")
    wt = nc.dram_tensor("wt", [NW, 128, NT, 128], BF16, kind="ExternalInput")
    bias = nc.dram_tensor("bias", [128, NW], F32, kind="ExternalInput")
    yt = nc.dram_tensor("yt", [OC, BC], BF16, kind="ExternalOutput")

    with tile.TileContext(nc) as tc, ExitStack() as ctx:
        xtp = ctx.enter_context(tc.tile_pool(name="xtp", bufs=1))
        wtp = ctx.enter_context(tc.tile_pool(name="wtp", bufs=2))
        bp = ctx.enter_context(tc.tile_pool(name="bp", bufs=1))
        op = ctx.enter_context(tc.tile_pool(name="op", bufs=4))
        pp = ctx.enter_context(tc.tile_pool(name="pp", bufs=8, space="PSUM"))

        loop = tc.For_i(0, reps, 1) if reps is not None else nullcontext()
        with loop:
            xt_res = xtp.tile([128, NT * BC], BF16)
            for t in range(NT):
                nc.scalar.dma_start(
                    xt_res[:, t * BC:(t + 1) * BC],
                    bass.AP(xt, t * 128 * BC, [[BC, 128], [1, BC]]),
                )
            bias_sb = bp.tile([128, NW], F32)
            nc.sync.dma_start(bias_sb[:], bass.AP(bias, 0, [[NW, 128], [1, NW]]))

            for w in range(NW):
                slab = wtp.tile([128, NT * 128], BF16)
                nc.sync.dma_start(
                    slab[:],
                    bass.AP(wt, w * 128 * NT * 128,
                            [[NT * 128, 128], [1, NT * 128]]),
                )
                psums = [pp.tile([128, 512], F32, name=f"ps{n}", tag="ps")
                         for n in range(NN)]
                for t in range(NT):
                    lhsT = slab[:, t * 128:(t + 1) * 128]
                    for n in range(NN):
                        nc.tensor.matmul(
                            psums[n][:],
                            lhsT,
                            xt_res[:, t * BC + n * 512: t * BC + n * 512 + 512],
                            start=(t == 0),
                            stop=(t == NT - 1),
                        )
                for n in range(NN):
                    ot = op.tile([128, 512], BF16)
                    nc.vector.tensor_scalar_add(ot[:], psums[n][:],
                                                bias_sb[:, w:w + 1])
                    nc.scalar.dma_start(
                        bass.AP(yt, w * 128 * BC + n * 512, [[BC, 128], [1, 512]]),
                        ot[:],
                    )
    _dedupe_ldweights(nc)
    nc.compile()
    return nc


def _build_probe_mm(reps=None, dtype="bf16", quad=False):
    """Pure-MM probe: x resident + ONE W slab, full 1024-MM schedule, no
    bias/output work (one token y write).  quad=True halves the LDW count
    by issuing 4 MMs per stationary (t range halved, rhs slices reused)."""
    import concourse.bacc as bacc
    import concourse.bass as bass
    import concourse.tile as tile
    from concourse import mybir
    from contextlib import ExitStack, nullcontext

    F32 = mybir.dt.float32
    MMD = mybir.dt.bfloat16 if dtype == "bf16" else mybir.dt.float32r
    OUTD = mybir.dt.bfloat16 if dtype == "bf16" else mybir.dt.float32

    nc = bacc.Bacc("TRN2", target_bir_lowering=False, debug=False)
    xt = nc.dram_tensor("xt", [NT, 128, BC], MMD, kind="ExternalInput")
    wt = nc.dram_tensor("wt", [NW, 128, NT, 128], MMD, kind="ExternalInput")
    bias = nc.dram_tensor("bias", [128, NW], F32, kind="ExternalInput")
    yt = nc.dram_tensor("yt", [OC, BC], OUTD, kind="ExternalOutput")

    if quad == "n256":
        NTP, NNP, NMOV = NT, 4, 256        # 2048 MMs of 256 cols
    elif quad:
        NTP, NNP, NMOV = NT // 2, 4, 512   # quad: half the t range
    else:
        NTP, NNP, NMOV = NT, NN, 512

    with tile.TileContext(nc) as tc, ExitStack() as ctx:
        xtp = ctx.enter_context(tc.tile_pool(name="xtp", bufs=1))
        wtp = ctx.enter_context(tc.tile_pool(name="wtp", bufs=1))
        bp = ctx.enter_context(tc.tile_pool(name="bp", bufs=1))
        op = ctx.enter_context(tc.tile_pool(name="op", bufs=1))
        pp = ctx.enter_context(tc.tile_pool(name="pp", bufs=8, space="PSUM"))

        loop = tc.For_i(0, reps, 1) if reps is not None else nullcontext()
        with loop:
            xt_res = xtp.tile([128, NT * BC], MMD)
            for t in range(NT):
                nc.scalar.dma_start(
                    xt_res[:, t * BC:(t + 1) * BC],
                    bass.AP(xt, t * 128 * BC, [[BC, 128], [1, BC]]),
                )
            bias_sb = bp.tile([128, NW], F32)
            nc.sync.dma_start(bias_sb[:], bass.AP(bias, 0, [[NW, 128], [1, NW]]))
            slab = wtp.tile([128, NT * 128], MMD)
            nc.sync.dma_start(
                slab[:], bass.AP(wt, 0, [[NT * 128, 128], [1, NT * 128]]))

            last_ps = None
            for w in range(NW):
                psums = [pp.tile([128, NMOV], F32, name=f"ps{n}", tag="ps")
                         for n in range(NNP)]
                for t in range(NTP):
                    lhsT = slab[:, t * 128:(t + 1) * 128]
                    for n in range(NNP):
                        col = (t * BC + (n * NMOV) % BC)
                        nc.tensor.matmul(
                            psums[n][:], lhsT, xt_res[:, col:col + NMOV],
                            start=(t == 0), stop=(t == NTP - 1),
                        )
                last_ps = psums[0]
            ot = op.tile([128, NMOV], OUTD)
            nc.vector.tensor_scalar_add(ot[:], last_ps[:], bias_sb[:, 0:1])
            nc.scalar.dma_start(
                bass.AP(yt, 0, [[BC, 128], [1, NMOV]]), ot[:])
    nc.compile()
    return nc


def _dedupe_ldweights(nc):
    """Remove redundant InstLdweights: tile_legalize emits one per matmul,
    but consecutive matmuls sharing the same stationary weights (our n=0/1
    pairs) only need the first.  The PE array retains the stationary operand,
    so an Ldweights identical (same physical AP) to the previous one on the
    PE queue -- with only Matmults in between and no semaphore waits of its
    own -- is a no-op that still costs ~128 serial cycles on HW."""
    from concourse import mybir

    removed = [0]

    def walk(block):
        insts = block.instructions
        last_sig = [None]

        keep = []
        for inst in insts:
            nm = type(inst).__name__
            if nm == "InstLdweights":
                si = inst.sync_info
                has_wait = si is not None and len(si.on_wait) > 0
                has_update = si is not None and len(si.on_update) > 0
                sig = (repr(inst.ins[0]), str(inst.perf_mode),
                       str(inst.is_transpose))
                if (not has_wait and not has_update
                        and sig == last_sig[0]):
                    removed[0] += 1
                    continue          # drop redundant reload
                last_sig[0] = sig
            elif nm == "InstMatmult":
                pass                  # keeps the loaded weights
            elif nm in ("InstEventSemaphore",):
                pass                  # sem-only, does not touch PE array
            else:
                last_sig[0] = None    # anything else: be conservative
            keep.append(inst)
            for sub in getattr(inst, "blocks", []) or []:
                walk(sub)
        if len(keep) != len(insts):
            while len(insts):
                insts.pop()
            for i in keep:
                insts.append(i)

    for b in nc.m.functions[0].blocks:
        walk(b)
    return removed[0]


def _build_program(reps=None, variant=None):
    if variant is None:
        variant = VARIANT
    if variant.startswith("probe_mm"):
        q = "n256" if "n256" in variant else ("quad" in variant)
        return _build_probe_mm(reps, dtype="bf16" if variant.endswith("bf16")
                               else "f32r", quad=q)
    if variant.startswith("bf16v2"):
        return _build_program_bf16v2(reps)
    if variant.startswith("bf16v3"):
        return _build_program_bf16v3(reps)
    if variant.startswith("bf16v4"):
        return _build_program_bf16v4(reps)
    dtype, pe_probe, dma_probe = _variant_opts(variant)

    import concourse.bacc as bacc
    import concourse.bass as bass
    import concourse.tile as tile
    from concourse import mybir
    from contextlib import ExitStack, nullcontext

    F32 = mybir.dt.float32
    MMD = mybir.dt.bfloat16 if dtype == "bf16" else mybir.dt.float32r
    OUTD = mybir.dt.bfloat16 if dtype == "bf16" else mybir.dt.float32

    nc = bacc.Bacc("TRN2", target_bir_lowering=False, debug=False)
    # host-tiled layouts (see _shard_inputs):
    #   xt[t, p, b]     = x[b0+b, 128t+p]
    #   wt[w, p, t, o'] = W[o0+128w+o', 128t+p]
    #   bias2[p, w]     = bias[o0+128w+p]
    xt = nc.dram_tensor("xt", [NT, 128, BC], MMD, kind="ExternalInput")
    wt = nc.dram_tensor("wt", [NW, 128, NT, 128], MMD, kind="ExternalInput")
    bias = nc.dram_tensor("bias", [128, NW], F32, kind="ExternalInput")
    yt = nc.dram_tensor("yt", [OC, BC], OUTD, kind="ExternalOutput")

    with tile.TileContext(nc) as tc, ExitStack() as ctx:
        xtp = ctx.enter_context(tc.tile_pool(name="xtp", bufs=1))
        wtp = ctx.enter_context(tc.tile_pool(name="wtp", bufs=2))
        bp = ctx.enter_context(tc.tile_pool(name="bp", bufs=1))
        op = ctx.enter_context(tc.tile_pool(name="op", bufs=4))
        pp = ctx.enter_context(tc.tile_pool(name="pp", bufs=8, space="PSUM"))

        loop = tc.For_i(0, reps, 1) if reps is not None else nullcontext()
        with loop:
            # resident x^T shard: [128, NT*BC] ; column block t holds j=128t+p
            xt_res = xtp.tile([128, NT * BC], MMD)
            for t in range(NT):
                nc.scalar.dma_start(
                    xt_res[:, t * BC:(t + 1) * BC],
                    bass.AP(xt, t * 128 * BC, [[BC, 128], [1, BC]]),
                )
            bias_sb = bp.tile([128, NW], F32)
            nc.sync.dma_start(bias_sb[:], bass.AP(bias, 0, [[NW, 128], [1, NW]]))

            shared_slab = None
            if pe_probe:
                shared_slab = wtp.tile([128, NT * 128], MMD, name="slab0")
                nc.sync.dma_start(
                    shared_slab[:],
                    bass.AP(wt, 0, [[NT * 128, 128], [1, NT * 128]]),
                )

            for w in range(NW):
                if pe_probe:
                    slab = shared_slab
                else:
                    slab = wtp.tile([128, NT * 128], MMD)
                    nc.sync.dma_start(
                        slab[:],
                        bass.AP(wt, w * 128 * NT * 128,
                                [[NT * 128, 128], [1, NT * 128]]),
                    )
                if dma_probe:
                    continue
                psums = [pp.tile([128, 512], F32, name=f"ps{n}", tag="ps")
                         for n in range(NN)]
                for t in range(NT):
                    lhsT = slab[:, t * 128:(t + 1) * 128]
                    for n in range(NN):
                        nc.tensor.matmul(
                            psums[n][:],
                            lhsT,
                            xt_res[:, t * BC + n * 512: t * BC + n * 512 + 512],
                            start=(t == 0),
                            stop=(t == NT - 1),
                        )
                for n in range(NN):
                    ot = op.tile([128, 512], OUTD)
                    nc.vector.tensor_scalar_add(ot[:], psums[n][:],
                                                bias_sb[:, w:w + 1])
                    nc.scalar.dma_start(
                        bass.AP(yt, w * 128 * BC + n * 512, [[BC, 128], [1, 512]]),
                        ot[:],
                    )
            if dma_probe:
                # keep the output-write traffic: DMA zeros-ish tiles out
                for w in range(NW):
                    for n in range(NN):
                        ot = op.tile([128, 512], OUTD)
                        nc.vector.memset(ot[:], 0.0)
                        nc.scalar.dma_start(
                            bass.AP(yt, w * 128 * BC + n * 512,
                                    [[BC, 128], [1, 512]]),
                            ot[:],
                        )
    nc.compile()
    return nc


def _get_program():
    global _PROGRAM
    if _PROGRAM is None:
        _PROGRAM = _build_program()
    return _PROGRAM


def _reconstruct_wt(upper_w: np.ndarray, lower_w: np.ndarray) -> np.ndarray:
    """Dense W [o, j] from the packed strict triangles (row-major fill)."""
    W = np.zeros((N, N), dtype=np.float32)
    iu = np.triu_indices(N, k=1)
    il = np.tril_indices(N, k=-1)
    W[iu] = upper_w
    W[il] = lower_w
    return W


def _shard_inputs(x, upper_w, lower_w, bias, variant=None):
    if variant is None:
        variant = VARIANT
    dtype, _, _ = _variant_opts(variant)
    x = np.asarray(x, dtype=np.float32)
    upper_w = np.asarray(upper_w, dtype=np.float32)
    lower_w = np.asarray(lower_w, dtype=np.float32)
    bias = np.asarray(bias, dtype=np.float32)

    W = _reconstruct_wt(upper_w, lower_w)

    if dtype == "bf16":
        import ml_dtypes
        mmdt = ml_dtypes.bfloat16
    else:
        mmdt = np.float32

    wt_shards = []
    bias_shards = []
    for ob in range(RO):
        Ws = W[ob * OC:(ob + 1) * OC, :]                       # [OC o, N j]
        # wt[w, p, t, o'] = Ws[128w+o', 128t+p]
        wts = np.ascontiguousarray(
            Ws.T.reshape(NT, 128, NW, 128).transpose(2, 1, 0, 3).astype(mmdt)
        )
        wt_shards.append(wts)
        bias_shards.append(
            np.ascontiguousarray(bias[ob * OC:(ob + 1) * OC].reshape(NW, 128).T)
        )

    xt_shards = []
    for bb in range(RB):
        xs = x[bb * BC:(bb + 1) * BC, :]                       # [BC b, N j]
        xt_shards.append(
            np.ascontiguousarray(xs.T.reshape(NT, 128, BC).astype(mmdt)))

    in_maps = []
    for c in range(8):
        ob, bb = c // RB, c % RB
        in_maps.append({
            "xt": xt_shards[bb],
            "wt": wt_shards[ob],
            "bias": bias_shards[ob],
        })
    return in_maps


def _assemble(results) -> np.ndarray:
    y = np.empty((N, N), dtype=np.float32)
    for c in range(8):
        ob, bb = c // RB, c % RB
        y[bb * BC:(bb + 1) * BC, ob * OC:(ob + 1) * OC] = \
            results[c]["yt"].T.astype(np.float32)
    return y


def kernel(x, upper_w, lower_w, bias):
    from concourse import bass_utils

    nc = _get_program()
    in_maps = _shard_inputs(x, upper_w, lower_w, bias)
    res = bass_utils.run_bass_kernel_spmd(nc, in_maps, core_ids=list(range(8)))
    return _assemble(res.results)
